# revision 1
# baseline (speedup 1.0000x reference)
"""Trainium2 Bass kernel for nn_PhysicsResidual (WavePINN wave-equation residual).

Per collocation point p = (t,x,y,z):
    u = MLP_128x6_tanh(p)   (4 -> 128 -> 128 x5 -> 1, tanh, linear head)
    psi = MLP_32x2_tanh(p)  (4 -> 32 -> 32 -> 1)
    d_i = diag(Hessian u)[i],  lap = d1+d2+d3
    resid = d0 - (1+psi)^2 * lap

Algorithm (per point, exact AD):
  forward:  h_k = tanh(a_k), a_k = W_k h_{k-1} + b_k, D_k = 1 - h_k^2
  backward: vt_6 = D_6*W_out^T, vt_{k-1} = D_{k-1}*(W_k^T vt_k)
            r_k = -2 * h_k * vt_k
  jets:     hdot_{1,i} = D_1 * W1[:,i]; adot_{k,i} = W_k hdot_{k-1,i};
            hdot_{k,i} = D_k * adot_{k,i}
  d_i = sum_k sum_j r_k[j] * adot_{k,i}[j]^2    (ones-matmul colsum,
        PSUM-accumulated over layers)

Matmuls in bf16 (4x faster than fp32 on PE; fp32 PSUM accumulate), except
the layer-1 / psi input matmuls which consume raw fp32 coordinates.
Sharding: data parallel, 16384 points -> 8 cores x 2048.
"""

import sys

sys.path.insert(0, "/opt/trn_rl_repo")

from contextlib import ExitStack

import numpy as np

import concourse.bacc as bacc
import concourse.bass as bass
import concourse.tile as tile
from concourse import mybir
from concourse.bass_utils import run_bass_kernel_spmd

N_CORES = 8
NPTS = 2048  # points per core
CHUNK = 512
NCHUNK = NPTS // CHUNK
W = 128  # WavePINN width
NHID = 5
NLAY = 6
PW = 32  # psi width

F32 = mybir.dt.float32
BF16 = mybir.dt.bfloat16
AF = mybir.ActivationFunctionType
ALU = mybir.AluOpType


def build_nc(stage="full"):
    nc = bacc.Bacc()

    pts = nc.declare_dram_parameter("pts", [4, NPTS], F32, isOutput=False)
    w1t = nc.declare_dram_parameter("w1t", [4, W], F32, isOutput=False)
    wfwd = nc.declare_dram_parameter("wfwd", [W, NHID * W], BF16, isOutput=False)
    wbwd = nc.declare_dram_parameter("wbwd", [W, NHID * W], BF16, isOutput=False)
    biases = nc.declare_dram_parameter("biases", [W, NLAY], F32, isOutput=False)
    wout = nc.declare_dram_parameter("wout", [W, 1], F32, isOutput=False)
    w1cols = nc.declare_dram_parameter("w1cols", [W, 4], F32, isOutput=False)
    jl = nc.declare_dram_parameter("jl", [W, 2], BF16, isOutput=False)
    ones = nc.declare_dram_parameter("ones", [W, 1], BF16, isOutput=False)
    pw1t = nc.declare_dram_parameter("pw1t", [4, PW], F32, isOutput=False)
    pw2t = nc.declare_dram_parameter("pw2t", [PW, PW], BF16, isOutput=False)
    pwot = nc.declare_dram_parameter("pwot", [PW, 1], BF16, isOutput=False)
    pbias = nc.declare_dram_parameter("pbias", [PW, 2], F32, isOutput=False)
    pb1 = nc.declare_dram_parameter("pb1", [1, 1], F32, isOutput=False)
    resid = nc.declare_dram_parameter("resid", [NCHUNK, CHUNK], F32, isOutput=True)

    with tile.TileContext(nc) as tc, ExitStack() as ctx:
        const = ctx.enter_context(tc.tile_pool(name="const", bufs=1))
        acts = ctx.enter_context(tc.tile_pool(name="acts", bufs=2))
        work = ctx.enter_context(tc.tile_pool(name="work", bufs=2))
        ps_a = ctx.enter_context(tc.tile_pool(name="ps_a", bufs=2, space="PSUM"))
        ps_j = ctx.enter_context(tc.tile_pool(name="ps_j", bufs=1, space="PSUM"))
        ps_d = ctx.enter_context(tc.tile_pool(name="ps_d", bufs=1, space="PSUM"))

        def load(name_ap, shape, tag, dt=F32):
            t = const.tile(shape, dt, tag=tag)
            nc.sync.dma_start(out=t[:], in_=name_ap[:])
            return t

        pts_sb = load(pts, [4, NPTS], "pts")
        w1t_sb = load(w1t, [4, W], "w1t")
        wfwd_sb = load(wfwd, [W, NHID * W], "wfwd", BF16)
        wbwd_sb = load(wbwd, [W, NHID * W], "wbwd", BF16)
        bias_sb = load(biases, [W, NLAY], "biases")
        wout_sb = load(wout, [W, 1], "wout")
        w1cols_sb = load(w1cols, [W, 4], "w1cols")
        jl_sb = load(jl, [W, 2], "jl", BF16)
        ones_sb = load(ones, [W, 1], "ones", BF16)
        pw1t_sb = load(pw1t, [4, PW], "pw1t")
        pw2t_sb = load(pw2t, [PW, PW], "pw2t", BF16)
        pwot_sb = load(pwot, [PW, 1], "pwot", BF16)
        pbias_sb = load(pbias, [PW, 2], "pbias")
        pb1_sb = load(pb1, [1, 1], "pb1")

        def wf(k):  # fwd lhsT for 0-idx layer k (1..5)
            return wfwd_sb[:, (k - 1) * W : k * W]

        def wb(k):  # bwd lhsT
            return wbwd_sb[:, (k - 1) * W : k * W]

        for c in range(NCHUNK):
            sl = slice(c * CHUNK, (c + 1) * CHUNK)

            y_sb = acts.tile([W, NLAY, CHUNK], BF16, tag="y")
            d_sb = acts.tile([W, NLAY, CHUNK], BF16, tag="d")
            r_sb = acts.tile([W, NLAY, CHUNK], BF16, tag="r")

            # ---- forward ----
            for k in range(NLAY):
                a_ps = ps_a.tile([W, CHUNK], F32, tag="a")
                if k == 0:
                    nc.tensor.matmul(a_ps, w1t_sb, pts_sb[:, sl], start=True, stop=True)
                else:
                    nc.tensor.matmul(a_ps, wf(k), y_sb[:, k - 1, :], start=True, stop=True)
                nc.scalar.activation(
                    y_sb[:, k, :], a_ps, AF.Tanh, bias=bias_sb[:, k : k + 1]
                )
                sq = work.tile([W, CHUNK], BF16, tag="sq")
                nc.gpsimd.tensor_tensor(sq, y_sb[:, k, :], y_sb[:, k, :], ALU.mult)
                nc.vector.tensor_scalar(
                    d_sb[:, k, :], sq, -1.0, 1.0, ALU.mult, ALU.add
                )

            if stage == "fwd":
                res_sb = work.tile([1, CHUNK], F32, tag="res")
                nc.vector.tensor_copy(res_sb, y_sb[0:1, NLAY - 1, :])
                nc.sync.dma_start(out=resid[c : c + 1, :], in_=res_sb[:])
                continue

            # ---- backward ----
            vt = work.tile([W, CHUNK], BF16, tag="vt")
            nc.vector.tensor_scalar_mul(vt, d_sb[:, NLAY - 1, :], wout_sb[:, 0:1])
            nc.vector.scalar_tensor_tensor(
                r_sb[:, NLAY - 1, :], y_sb[:, NLAY - 1, :], -2.0, vt, ALU.mult, ALU.mult
            )
            for k in range(NLAY - 1, 0, -1):
                v_ps = ps_a.tile([W, CHUNK], F32, tag="a")
                nc.tensor.matmul(v_ps, wb(k), vt, start=True, stop=True)
                vt = work.tile([W, CHUNK], BF16, tag="vt")
                nc.vector.tensor_tensor(vt, d_sb[:, k - 1, :], v_ps, ALU.mult)
                nc.vector.scalar_tensor_tensor(
                    r_sb[:, k - 1, :], y_sb[:, k - 1, :], -2.0, vt, ALU.mult, ALU.mult
                )

            if stage == "bwd":
                res_sb = work.tile([1, CHUNK], F32, tag="res")
                nc.vector.tensor_copy(res_sb, r_sb[0:1, 0, :])
                nc.sync.dma_start(out=resid[c : c + 1, :], in_=res_sb[:])
                continue

            # ---- jets + curvature contraction ----
            dlap_ps = ps_d.tile([1, CHUNK], F32, tag="dlap")
            dt_ps = ps_d.tile([1, CHUNK], F32, tag="dt")
            nc.tensor.matmul(
                dlap_ps, jl_sb[:, 0:1], r_sb[:, 0, :], start=True, stop=False,
                skip_group_check=True,
            )
            nc.tensor.matmul(
                dt_ps, jl_sb[:, 1:2], r_sb[:, 0, :], start=True, stop=False,
                skip_group_check=True,
            )
            hj = work.tile([W, 4, CHUNK], BF16, tag="hj")
            for i in range(4):
                nc.vector.tensor_scalar_mul(
                    hj[:, i, :], d_sb[:, 0, :], w1cols_sb[:, i : i + 1]
                )
            for k in range(1, NLAY):
                aj_ps = ps_j.tile([W, 4, CHUNK], F32, tag="aj")
                for i in range(4):
                    nc.tensor.matmul(
                        aj_ps[:, i, :], wf(k), hj[:, i, :], start=True, stop=True
                    )
                sqj = work.tile([W, 4, CHUNK], BF16, tag="sqj")
                nc.scalar.activation(sqj, aj_ps, AF.Square)
                if k < NLAY - 1:
                    hj = work.tile([W, 4, CHUNK], BF16, tag="hj")
                    for i in range(4):
                        nc.vector.tensor_tensor(
                            hj[:, i, :], d_sb[:, k, :], aj_ps[:, i, :], ALU.mult
                        )
                p_sum = work.tile([W, CHUNK], BF16, tag="p_sum")
                nc.vector.tensor_tensor(p_sum, sqj[:, 1, :], sqj[:, 2, :], ALU.add)
                nc.vector.tensor_tensor(p_sum, p_sum, sqj[:, 3, :], ALU.add)
                ulap = work.tile([W, CHUNK], BF16, tag="ulap")
                nc.vector.tensor_tensor(ulap, p_sum, r_sb[:, k, :], ALU.mult)
                ut = work.tile([W, CHUNK], BF16, tag="ut")
                nc.vector.tensor_tensor(ut, sqj[:, 0, :], r_sb[:, k, :], ALU.mult)
                last = k == NLAY - 1
                nc.tensor.matmul(
                    dlap_ps, ones_sb, ulap, start=False, stop=last,
                    skip_group_check=True,
                )
                nc.tensor.matmul(
                    dt_ps, ones_sb, ut, start=False, stop=last,
                    skip_group_check=True,
                )

            if stage == "jets":
                res_sb = work.tile([1, CHUNK], F32, tag="res")
                nc.vector.tensor_copy(res_sb, dt_ps)
                nc.sync.dma_start(out=resid[c : c + 1, :], in_=res_sb[:])
                continue

            # ---- psi network ----
            pp_ps = ps_a.tile([PW, CHUNK], F32, tag="a")
            nc.tensor.matmul(pp_ps, pw1t_sb, pts_sb[:, sl], start=True, stop=True)
            hp1 = work.tile([PW, CHUNK], BF16, tag="hp")
            nc.scalar.activation(hp1, pp_ps, AF.Tanh, bias=pbias_sb[:, 0:1])
            pp2_ps = ps_a.tile([PW, CHUNK], F32, tag="a")
            nc.tensor.matmul(pp2_ps, pw2t_sb, hp1, start=True, stop=True)
            hp2 = work.tile([PW, CHUNK], BF16, tag="hp")
            nc.scalar.activation(hp2, pp2_ps, AF.Tanh, bias=pbias_sb[:, 1:2])
            psi_ps = ps_a.tile([1, CHUNK], F32, tag="a")
            nc.tensor.matmul(psi_ps, pwot_sb, hp2, start=True, stop=True)
            c2 = work.tile([1, CHUNK], F32, tag="c2")
            nc.scalar.activation(c2, psi_ps, AF.Square, bias=pb1_sb[:, 0:1])

            # ---- tail: resid = dt - c2*dlap ----
            dl_sb = work.tile([1, CHUNK], F32, tag="dl")
            nc.vector.tensor_copy(dl_sb, dlap_ps)
            dt_sb = work.tile([1, CHUNK], F32, tag="dtb")
            nc.vector.tensor_copy(dt_sb, dt_ps)
            m1 = work.tile([1, CHUNK], F32, tag="m1")
            nc.vector.scalar_tensor_tensor(m1, dl_sb, -1.0, c2, ALU.mult, ALU.mult)
            res_sb = work.tile([1, CHUNK], F32, tag="res")
            nc.vector.tensor_tensor(res_sb, m1, dt_sb, ALU.add)
            nc.sync.dma_start(out=resid[c : c + 1, :], in_=res_sb[:])

    return nc


_NC_CACHE = {}


def _get_nc():
    if "nc" not in _NC_CACHE:
        nc = build_nc()
        nc.finalize()
        _NC_CACHE["nc"] = nc
    return _NC_CACHE["nc"]


def _bf(a):
    import ml_dtypes

    return np.asarray(a, np.float32).astype(ml_dtypes.bfloat16)


def make_in_maps(t, x, y, z, uW_in, ub_in, uW_hid, ub_hid, uW_out, ub_out,
                 pW_in, pb_in, pW_hid, pb_hid, pW_out, pb_out):
    f = lambda a: np.ascontiguousarray(np.asarray(a, np.float32))
    uW_in, ub_in, uW_hid, ub_hid = f(uW_in), f(ub_in), f(uW_hid), f(ub_hid)
    uW_out, pW_in, pb_in = f(uW_out), f(pW_in), f(pb_in)
    pW_hid, pb_hid, pW_out, pb_out = f(pW_hid), f(pb_hid), f(pW_out), f(pb_out)

    pts_full = np.stack([f(t), f(x), f(y), f(z)], axis=0)  # [4, 16384]

    shared = dict(
        w1t=f(uW_in.T),
        wfwd=_bf(np.concatenate([uW_hid[i].T for i in range(NHID)], axis=1)),
        wbwd=_bf(np.concatenate([uW_hid[i] for i in range(NHID)], axis=1)),
        biases=f(np.concatenate([ub_in[:, None], ub_hid.T], axis=1)),
        wout=f(uW_out[0][:, None]),
        w1cols=uW_in.copy(),
        jl=_bf(np.stack([(uW_in[:, 1:4] ** 2).sum(1), uW_in[:, 0] ** 2], axis=1)),
        ones=_bf(np.ones([W, 1], np.float32)),
        pw1t=f(pW_in.T),
        pw2t=_bf(pW_hid[0].T),
        pwot=_bf(pW_out[0][:, None]),
        pbias=f(np.stack([pb_in, pb_hid[0]], axis=1)),
        pb1=np.array([[pb_out[0] + 1.0]], np.float32),
    )
    in_maps = []
    for cid in range(N_CORES):
        m = dict(shared)
        m["pts"] = np.ascontiguousarray(pts_full[:, cid * NPTS : (cid + 1) * NPTS])
        in_maps.append(m)
    return in_maps


def kernel(**inputs):
    in_maps = make_in_maps(**inputs)
    nc = _get_nc()
    res = run_bass_kernel_spmd(nc, in_maps, list(range(N_CORES))).results
    out = np.concatenate(
        [np.asarray(res[cid]["resid"]).reshape(-1) for cid in range(N_CORES)]
    )
    return out.astype(np.float32)


if __name__ == "__main__":
    nc = build_nc()
    print("built ok:", nc)



# revision 3
# speedup vs baseline: 1.2560x; 1.2560x over previous
"""Trainium2 Bass kernel for nn_PhysicsResidual (WavePINN wave-equation residual).

Per collocation point p = (t,x,y,z):
    u = MLP_128x6_tanh(p)   (4 -> 128 -> 128 x5 -> 1, tanh, linear head)
    psi = MLP_32x2_tanh(p)  (4 -> 32 -> 32 -> 1)
    d_i = diag(Hessian u)[i],  lap = d1+d2+d3
    resid = d0 - (1+psi)^2 * lap

Algorithm (per point, exact AD):
  forward:  h_k = tanh(a_k), a_k = W_k h_{k-1} + b_k, D_k = 1 - h_k^2
  backward: vt_6 = D_6*W_out^T, vt_{k-1} = D_{k-1}*(W_k^T vt_k)
            r_k = -2 * h_k * vt_k
  jets:     hdot_{1,i} = D_1 * W1[:,i]; adot_{k,i} = W_k hdot_{k-1,i};
            hdot_{k,i} = D_k * adot_{k,i}
  d_i = sum_k sum_j r_k[j] * adot_{k,i}[j]^2    (ones-matmul colsum,
        PSUM-accumulated over layers; dt row 0, dlap row 32 of acc tile)

Engine split: ACT = tanh + jets squares (PSUM drains); DVE = backward vt,
jets D-mult and r-weighted products (broadcast APs); GpSimd = h^2 and
r = -2*h*vt.  Jets run as two independent 2-direction pipelines (dirs (t,x)
and (y,z)) with double-buffered [W,2,C] PSUM tiles so consecutive layers and
chunks overlap.
Sharding: data parallel, 16384 points -> 8 cores x 2048.
"""

import sys

sys.path.insert(0, "/opt/trn_rl_repo")

from contextlib import ExitStack

import numpy as np

import concourse.bacc as bacc
import concourse.bass as bass
import concourse.tile as tile
from concourse import mybir
from concourse.bass_utils import run_bass_kernel_spmd

N_CORES = 8
NPTS = 2048  # points per core
CHUNK = 512
NCHUNK = NPTS // CHUNK
W = 128  # WavePINN width
NHID = 5
NLAY = 6
PW = 32  # psi width

F32 = mybir.dt.float32
BF16 = mybir.dt.bfloat16
AF = mybir.ActivationFunctionType
ALU = mybir.AluOpType


def build_nc(stage="full"):
    nc = bacc.Bacc()

    pts = nc.declare_dram_parameter("pts", [4, NPTS], F32, isOutput=False)
    # bundled weights: fewer DMA dispatches at startup
    wpf = nc.declare_dram_parameter("wpf", [W, 11], F32, isOutput=False)
    wpb = nc.declare_dram_parameter("wpb", [W, 2 * NHID * W + 3], BF16, isOutput=False)
    ppack = nc.declare_dram_parameter("ppack", [4, W + PW], F32, isOutput=False)
    pwpack = nc.declare_dram_parameter("pwpack", [PW, PW + 1], BF16, isOutput=False)
    pbias3 = nc.declare_dram_parameter("pbias3", [PW, 3], F32, isOutput=False)
    resid = nc.declare_dram_parameter("resid", [NCHUNK, CHUNK], F32, isOutput=True)

    with tile.TileContext(nc) as tc, ExitStack() as ctx:
        const = ctx.enter_context(tc.tile_pool(name="const", bufs=1))
        acts = ctx.enter_context(tc.tile_pool(name="acts", bufs=3))
        work = ctx.enter_context(tc.tile_pool(name="work", bufs=2))
        jwork = ctx.enter_context(tc.tile_pool(name="jwork", bufs=3))
        ps_f = ctx.enter_context(tc.tile_pool(name="ps_f", bufs=2, space="PSUM"))
        ps_j = ctx.enter_context(tc.tile_pool(name="ps_j", bufs=2, space="PSUM"))
        ps_d = ctx.enter_context(tc.tile_pool(name="ps_d", bufs=2, space="PSUM"))

        # ---- const loads spread over engine queues (each queue serializes
        # its own DMAs); pts split per-chunk so chunk 0 starts early ----
        ppack_sb = const.tile([4, W + PW], F32, tag="ppack")
        nc.gpsimd.dma_start(out=ppack_sb[:], in_=ppack[:])
        wpb_sb = const.tile([W, 2 * NHID * W + 3], BF16, tag="wpb")
        nc.gpsimd.dma_start(out=wpb_sb[:], in_=wpb[:])
        wpf_sb = const.tile([W, 11], F32, tag="wpf")
        nc.scalar.dma_start(out=wpf_sb[:], in_=wpf[:])
        pwpack_sb = const.tile([PW, PW + 1], BF16, tag="pwpack")
        nc.scalar.dma_start(out=pwpack_sb[:], in_=pwpack[:])
        pts_sb = const.tile([4, NPTS], F32, tag="pts")
        for c in range(NCHUNK):
            nc.sync.dma_start(
                out=pts_sb[:, c * CHUNK : (c + 1) * CHUNK],
                in_=pts[:, c * CHUNK : (c + 1) * CHUNK],
            )
        pbias3_sb = const.tile([PW, 3], F32, tag="pbias3")
        nc.sync.dma_start(out=pbias3_sb[:], in_=pbias3[:])

        # ---- warmup: pull the ACT table load + PE clock ramp off the
        # critical path while the input DMAs are in flight ----
        wz = work.tile([W, CHUNK], BF16, tag="wz")
        nc.gpsimd.memset(wz, 0)
        wrm = work.tile([1, 1], F32, tag="wrm")
        nc.gpsimd.memset(wrm, 0)
        nc.scalar.activation(wrm, wrm, AF.Tanh)
        for _ in range(6):
            wm_ps = ps_f.tile([W, CHUNK], F32, tag="a")
            nc.tensor.matmul(wm_ps, wz[:, 0:W], wz, start=True, stop=True)

        bias_sb = wpf_sb[:, 0:NLAY]
        wout_sb = wpf_sb[:, NLAY : NLAY + 1]
        w1cols_sb = wpf_sb[:, NLAY + 1 : NLAY + 5]
        wfwd_sb = wpb_sb[:, 0 : NHID * W]
        wbwd_sb = wpb_sb[:, NHID * W : 2 * NHID * W]
        jl2_sb = wpb_sb[:, 2 * NHID * W : 2 * NHID * W + 2]
        ones_sb = wpb_sb[:, 2 * NHID * W + 2 : 2 * NHID * W + 3]
        w1t_sb = ppack_sb[:, 0:W]
        pw1t_sb = ppack_sb[:, W : W + PW]
        pw2t_sb = pwpack_sb[:, 0:PW]
        pwot_sb = pwpack_sb[:, PW : PW + 1]
        pb1_sb = pbias3_sb[0:1, 2:3]

        def wf(k):  # fwd lhsT for 0-idx layer k (1..5)
            return wfwd_sb[:, (k - 1) * W : k * W]

        def wb(k):  # bwd lhsT
            return wbwd_sb[:, (k - 1) * W : k * W]

        for c in range(NCHUNK):
            sl = slice(c * CHUNK, (c + 1) * CHUNK)

            y_sb = acts.tile([W, NLAY, CHUNK], BF16, tag="y")
            d_sb = acts.tile([W, NLAY, CHUNK], BF16, tag="d")
            r_sb = acts.tile([W, NLAY, CHUNK], BF16, tag="r")

            # ---- forward ----
            for k in range(NLAY):
                a_ps = ps_f.tile([W, CHUNK], F32, tag="a")
                if k == 0:
                    nc.tensor.matmul(a_ps, w1t_sb, pts_sb[:, sl], start=True, stop=True)
                else:
                    nc.tensor.matmul(a_ps, wf(k), y_sb[:, k - 1, :], start=True, stop=True)
                nc.scalar.activation(
                    y_sb[:, k, :], a_ps, AF.Tanh, bias=bias_sb[:, k : k + 1]
                )
                sq = work.tile([W, CHUNK], BF16, tag="sq")
                nc.gpsimd.tensor_tensor(sq, y_sb[:, k, :], y_sb[:, k, :], ALU.mult)
                nc.vector.tensor_scalar(
                    d_sb[:, k, :], sq, -1.0, 1.0, ALU.mult, ALU.add
                )

            if stage == "fwd":
                res_sb = work.tile([1, CHUNK], F32, tag="res")
                nc.vector.tensor_copy(res_sb, y_sb[0:1, NLAY - 1, :])
                nc.sync.dma_start(out=resid[c : c + 1, :], in_=res_sb[:])
                continue

            # ---- backward ----
            vt = work.tile([W, CHUNK], BF16, tag="vt")
            nc.vector.tensor_scalar_mul(vt, d_sb[:, NLAY - 1, :], wout_sb)
            nc.gpsimd.tensor_tensor(
                r_sb[:, NLAY - 1, :], y_sb[:, NLAY - 1, :], vt, ALU.mult
            )
            for k in range(NLAY - 1, 0, -1):
                v_ps = ps_f.tile([W, CHUNK], F32, tag="a")
                nc.tensor.matmul(v_ps, wb(k), vt, start=True, stop=True)
                vt = work.tile([W, CHUNK], BF16, tag="vt")
                nc.vector.tensor_tensor(vt, d_sb[:, k - 1, :], v_ps, ALU.mult)
                nc.gpsimd.tensor_tensor(
                    r_sb[:, k - 1, :], y_sb[:, k - 1, :], vt, ALU.mult
                )

            if stage == "bwd":
                res_sb = work.tile([1, CHUNK], F32, tag="res")
                nc.vector.tensor_copy(res_sb, r_sb[0:1, 0, :])
                nc.sync.dma_start(out=resid[c : c + 1, :], in_=res_sb[:])
                continue

            # ---- psi network (independent; scheduler slots it into gaps) ----
            pp_ps = ps_f.tile([PW, CHUNK], F32, tag="a")
            nc.tensor.matmul(pp_ps, pw1t_sb, pts_sb[:, sl], start=True, stop=True)
            hp1 = work.tile([PW, CHUNK], BF16, tag="hp")
            nc.scalar.activation(hp1, pp_ps, AF.Tanh, bias=pbias3_sb[:, 0:1])
            pp2_ps = ps_f.tile([PW, CHUNK], F32, tag="a")
            nc.tensor.matmul(pp2_ps, pw2t_sb, hp1, start=True, stop=True)
            hp2 = work.tile([PW, CHUNK], BF16, tag="hp")
            nc.scalar.activation(hp2, pp2_ps, AF.Tanh, bias=pbias3_sb[:, 1:2])
            psi_ps = ps_f.tile([1, CHUNK], F32, tag="a")
            nc.tensor.matmul(psi_ps, pwot_sb, hp2, start=True, stop=True)
            c2 = work.tile([1, CHUNK], F32, tag="c2")
            nc.scalar.activation(c2, psi_ps, AF.Square, bias=pb1_sb)

            # ---- jets + curvature contraction ----
            # acc row 0 = u_tt, row 32 = lap (matmul outs need base part 0/32/64)
            # two independent direction-pair pipelines: half 0 = (t,x), 1 = (y,z)
            acc_ps = ps_d.tile([33, CHUNK], F32, tag="acc")
            nc.tensor.matmul(
                acc_ps[0:1, :], jl2_sb[:, 0:1], r_sb[:, 0, :], start=True, stop=False,
                skip_group_check=True,
            )
            nc.tensor.matmul(
                acc_ps[32:33, :], jl2_sb[:, 1:2], r_sb[:, 0, :], start=True, stop=False,
                skip_group_check=True,
            )
            hjh = []
            for h in range(2):
                hj = jwork.tile([W, 2, CHUNK], BF16, tag="hj")
                for i in range(2):
                    nc.vector.tensor_scalar_mul(
                        hj[:, i, :], d_sb[:, 0, :],
                        w1cols_sb[:, 2 * h + i : 2 * h + i + 1],
                    )
                hjh.append(hj)
            for k in range(1, NLAY):
                last = k == NLAY - 1
                for h in range(2):
                    aj_ps = ps_j.tile([W, 2, CHUNK], F32, tag="aj")
                    for i in range(2):
                        nc.tensor.matmul(
                            aj_ps[:, i, :], wf(k), hjh[h][:, i, :],
                            start=True, stop=True,
                        )
                    sqj = jwork.tile([W, 2, CHUNK], BF16, tag="sqj")
                    nc.scalar.activation(sqj, aj_ps, AF.Square)
                    if not last:
                        hj = jwork.tile([W, 2, CHUNK], BF16, tag="hj")
                        dbc = d_sb[:, k, :].unsqueeze(1).to_broadcast((W, 2, CHUNK))
                        nc.vector.tensor_tensor(hj, dbc, aj_ps, ALU.mult)
                        hjh[h] = hj
                    # mall[:,i,:] = sqj[:,i,:] * r_k  (fused, r broadcast)
                    mall = jwork.tile([W, 2, CHUNK], BF16, tag="mall")
                    rbc = r_sb[:, k, :].unsqueeze(1).to_broadcast((W, 2, CHUNK))
                    nc.vector.tensor_tensor(mall, sqj, rbc, ALU.mult)
                    if h == 0:
                        nc.tensor.matmul(
                            acc_ps[0:1, :], ones_sb, mall[:, 0, :],
                            start=False, stop=last, skip_group_check=True,
                        )
                        nc.tensor.matmul(
                            acc_ps[32:33, :], ones_sb, mall[:, 1, :],
                            start=False, stop=False, skip_group_check=True,
                        )
                    else:
                        for i in range(2):
                            nc.tensor.matmul(
                                acc_ps[32:33, :], ones_sb, mall[:, i, :],
                                start=False, stop=(last and i == 1),
                                skip_group_check=True,
                            )

            if stage == "jets":
                res_sb = work.tile([1, CHUNK], F32, tag="res")
                nc.vector.tensor_copy(res_sb, acc_ps[0:1, :])
                nc.sync.dma_start(out=resid[c : c + 1, :], in_=res_sb[:])
                continue

            # ---- tail: resid = dt - c2*dlap ----
            m1 = work.tile([1, CHUNK], F32, tag="m1")
            nc.vector.scalar_tensor_tensor(
                m1, acc_ps[32:33, :], -1.0, c2, ALU.mult, ALU.mult
            )
            res_sb = work.tile([1, CHUNK], F32, tag="res")
            nc.vector.tensor_tensor(res_sb, m1, acc_ps[0:1, :], ALU.add)
            nc.sync.dma_start(out=resid[c : c + 1, :], in_=res_sb[:])

    return nc


_NC_CACHE = {}


def _get_nc():
    if "nc" not in _NC_CACHE:
        nc = build_nc()
        nc.finalize()
        _NC_CACHE["nc"] = nc
    return _NC_CACHE["nc"]


def _bf(a):
    import ml_dtypes

    return np.asarray(a, np.float32).astype(ml_dtypes.bfloat16)


def make_in_maps(t, x, y, z, uW_in, ub_in, uW_hid, ub_hid, uW_out, ub_out,
                 pW_in, pb_in, pW_hid, pb_hid, pW_out, pb_out):
    f = lambda a: np.ascontiguousarray(np.asarray(a, np.float32))
    uW_in, ub_in, uW_hid, ub_hid = f(uW_in), f(ub_in), f(uW_hid), f(ub_hid)
    uW_out, pW_in, pb_in = f(uW_out), f(pW_in), f(pb_in)
    pW_hid, pb_hid, pW_out, pb_out = f(pW_hid), f(pb_hid), f(pW_out), f(pb_out)

    pts_full = np.stack([f(t), f(x), f(y), f(z)], axis=0)  # [4, 16384]

    # wpf: biases [W,6] | wout [W,1] | w1cols [W,4]
    biases = np.concatenate([ub_in[:, None], ub_hid.T], axis=1)
    wpf = np.concatenate([biases, uW_out[0][:, None], uW_in], axis=1)
    # wpb: wfwd [W,640] | wbwd [W,640] | jl2 [W,2] | ones [W,1]
    wfwd = np.concatenate([uW_hid[i].T for i in range(NHID)], axis=1)
    wbwd = np.concatenate([uW_hid[i] for i in range(NHID)], axis=1)
    jl2 = -2.0 * np.stack([uW_in[:, 0] ** 2, (uW_in[:, 1:4] ** 2).sum(1)], axis=1)
    wpb = np.concatenate([wfwd, wbwd, jl2, -2.0 * np.ones([W, 1], np.float32)], axis=1)
    # ppack: w1t [4,128] | pw1t [4,32]
    ppk = np.concatenate([uW_in.T, pW_in.T], axis=1)
    # pwpack: pw2t [32,32] | pwot [32,1]
    pwp = np.concatenate([pW_hid[0].T, pW_out[0][:, None]], axis=1)
    # pbias3: pb_in | pb_hid[0] | (pb_out+1 at row 0)
    pb3 = np.zeros([PW, 3], np.float32)
    pb3[:, 0] = pb_in
    pb3[:, 1] = pb_hid[0]
    pb3[0, 2] = pb_out[0] + 1.0

    shared = dict(
        wpf=f(wpf),
        wpb=_bf(wpb),
        ppack=f(ppk),
        pwpack=_bf(pwp),
        pbias3=f(pb3),
    )
    in_maps = []
    for cid in range(N_CORES):
        m = dict(shared)
        m["pts"] = np.ascontiguousarray(pts_full[:, cid * NPTS : (cid + 1) * NPTS])
        in_maps.append(m)
    return in_maps


def kernel(**inputs):
    in_maps = make_in_maps(**inputs)
    nc = _get_nc()
    res = run_bass_kernel_spmd(nc, in_maps, list(range(N_CORES))).results
    out = np.concatenate(
        [np.asarray(res[cid]["resid"]).reshape(-1) for cid in range(N_CORES)]
    )
    return out.astype(np.float32)


if __name__ == "__main__":
    nc = build_nc()
    print("built ok:", nc)


# revision 4
# speedup vs baseline: 1.2696x; 1.0108x over previous
"""Trainium2 Bass kernel for nn_PhysicsResidual (WavePINN wave-equation residual).

Per collocation point p = (t,x,y,z):
    u = MLP_128x6_tanh(p)   (4 -> 128 -> 128 x5 -> 1, tanh, linear head)
    psi = MLP_32x2_tanh(p)  (4 -> 32 -> 32 -> 1)
    d_i = diag(Hessian u)[i],  lap = d1+d2+d3
    resid = d0 - (1+psi)^2 * lap

Algorithm (per point, exact AD):
  forward:  h_k = tanh(a_k), a_k = W_k h_{k-1} + b_k, D_k = 1 - h_k^2
  backward: vt_6 = D_6*W_out^T, vt_{k-1} = D_{k-1}*(W_k^T vt_k)
            r_k = -2 * h_k * vt_k
  jets:     hdot_{1,i} = D_1 * W1[:,i]; adot_{k,i} = W_k hdot_{k-1,i};
            hdot_{k,i} = D_k * adot_{k,i}
  d_i = sum_k sum_j r_k[j] * adot_{k,i}[j]^2    (ones-matmul colsum,
        PSUM-accumulated over layers; dt row 0, dlap row 32 of acc tile)

Engine split: ACT = tanh + jets squares (PSUM drains); DVE = backward vt,
jets D-mult and r-weighted products (broadcast APs); GpSimd = h^2 and
r = -2*h*vt.  Jets run as two independent 2-direction pipelines (dirs (t,x)
and (y,z)) with double-buffered [W,2,C] PSUM tiles so consecutive layers and
chunks overlap.
Sharding: data parallel, 16384 points -> 8 cores x 2048.
"""

import sys

sys.path.insert(0, "/opt/trn_rl_repo")

from contextlib import ExitStack

import numpy as np

import concourse.bacc as bacc
import concourse.bass as bass
import concourse.tile as tile
from concourse import mybir
from concourse.bass_utils import run_bass_kernel_spmd

N_CORES = 8
NPTS = 2048  # points per core
CHUNK = 512
NCHUNK = NPTS // CHUNK
W = 128  # WavePINN width
NHID = 5
NLAY = 6
PW = 32  # psi width

F32 = mybir.dt.float32
BF16 = mybir.dt.bfloat16
AF = mybir.ActivationFunctionType
ALU = mybir.AluOpType


def build_nc(stage="full"):
    nc = bacc.Bacc()

    pts = nc.declare_dram_parameter("pts", [4, NPTS], F32, isOutput=False)
    # bundled weights: fewer DMA dispatches at startup
    wpf = nc.declare_dram_parameter("wpf", [W, 11], F32, isOutput=False)
    wpb = nc.declare_dram_parameter("wpb", [W, 2 * NHID * W + 3], BF16, isOutput=False)
    ppack = nc.declare_dram_parameter("ppack", [4, W + PW], F32, isOutput=False)
    pwpack = nc.declare_dram_parameter("pwpack", [PW, PW + 1], BF16, isOutput=False)
    pbias3 = nc.declare_dram_parameter("pbias3", [PW, 3], F32, isOutput=False)
    resid = nc.declare_dram_parameter("resid", [NCHUNK, CHUNK], F32, isOutput=True)

    with tile.TileContext(nc) as tc, ExitStack() as ctx:
        const = ctx.enter_context(tc.tile_pool(name="const", bufs=1))
        acts = ctx.enter_context(tc.tile_pool(name="acts", bufs=3))
        work = ctx.enter_context(tc.tile_pool(name="work", bufs=2))
        jwork = ctx.enter_context(tc.tile_pool(name="jwork", bufs=3))
        ps_f = ctx.enter_context(tc.tile_pool(name="ps_f", bufs=2, space="PSUM"))
        ps_j = ctx.enter_context(tc.tile_pool(name="ps_j", bufs=2, space="PSUM"))
        ps_d = ctx.enter_context(tc.tile_pool(name="ps_d", bufs=1, space="PSUM"))
        ps_hb = ctx.enter_context(tc.tile_pool(name="ps_hb", bufs=1, space="PSUM"))

        # ---- const loads spread over engine queues (each queue serializes
        # its own DMAs); pts split per-chunk so chunk 0 starts early ----
        ppack_sb = const.tile([4, W + PW], F32, tag="ppack")
        nc.gpsimd.dma_start(out=ppack_sb[:], in_=ppack[:])
        wpb_sb = const.tile([W, 2 * NHID * W + 3], BF16, tag="wpb")
        nc.gpsimd.dma_start(out=wpb_sb[:], in_=wpb[:])
        wpf_sb = const.tile([W, 11], F32, tag="wpf")
        nc.scalar.dma_start(out=wpf_sb[:], in_=wpf[:])
        pwpack_sb = const.tile([PW, PW + 1], BF16, tag="pwpack")
        nc.scalar.dma_start(out=pwpack_sb[:], in_=pwpack[:])
        pts_sb = const.tile([4, NPTS], F32, tag="pts")
        for c in range(NCHUNK):
            nc.sync.dma_start(
                out=pts_sb[:, c * CHUNK : (c + 1) * CHUNK],
                in_=pts[:, c * CHUNK : (c + 1) * CHUNK],
            )
        pbias3_sb = const.tile([PW, 3], F32, tag="pbias3")
        nc.sync.dma_start(out=pbias3_sb[:], in_=pbias3[:])

        # ---- warmup: pull the ACT table load + PE clock ramp off the
        # critical path while the input DMAs are in flight ----
        wz = work.tile([W, CHUNK], BF16, tag="wz")
        nc.gpsimd.memset(wz, 0)
        wrm = work.tile([1, 1], F32, tag="wrm")
        nc.gpsimd.memset(wrm, 0)
        nc.scalar.activation(wrm, wrm, AF.Tanh)
        hb_ps = ps_hb.tile([W, CHUNK], F32, tag="hb")
        for _ in range(9):
            nc.tensor.matmul(hb_ps, wz[:, 0:W], wz, start=True, stop=True)

        def heartbeat(lhsT, rhs):
            # keep the PE activity monitor warm: dummy matmul tied to a
            # freshly produced tensor so the scheduler spreads them in time;
            # reuses the stationary weights of the neighboring real matmul
            nc.tensor.matmul(
                hb_ps[:, 0:256], lhsT, rhs[:, 0:256], start=True, stop=True,
            )

        bias_sb = wpf_sb[:, 0:NLAY]
        wout_sb = wpf_sb[:, NLAY : NLAY + 1]
        w1cols_sb = wpf_sb[:, NLAY + 1 : NLAY + 5]
        wfwd_sb = wpb_sb[:, 0 : NHID * W]
        wbwd_sb = wpb_sb[:, NHID * W : 2 * NHID * W]
        jl2_sb = wpb_sb[:, 2 * NHID * W : 2 * NHID * W + 2]
        ones_sb = wpb_sb[:, 2 * NHID * W + 2 : 2 * NHID * W + 3]
        w1t_sb = ppack_sb[:, 0:W]
        pw1t_sb = ppack_sb[:, W : W + PW]
        pw2t_sb = pwpack_sb[:, 0:PW]
        pwot_sb = pwpack_sb[:, PW : PW + 1]
        pb1_sb = pbias3_sb[0:1, 2:3]

        def wf(k):  # fwd lhsT for 0-idx layer k (1..5)
            return wfwd_sb[:, (k - 1) * W : k * W]

        def wb(k):  # bwd lhsT
            return wbwd_sb[:, (k - 1) * W : k * W]

        for c in range(NCHUNK):
            sl = slice(c * CHUNK, (c + 1) * CHUNK)

            y_sb = acts.tile([W, NLAY, CHUNK], BF16, tag="y")
            d_sb = acts.tile([W, NLAY, CHUNK], BF16, tag="d")
            r_sb = acts.tile([W, NLAY, CHUNK], BF16, tag="r")

            # ---- forward ----
            for k in range(NLAY):
                a_ps = ps_f.tile([W, CHUNK], F32, tag="a")
                if k == 0:
                    nc.tensor.matmul(a_ps, w1t_sb, pts_sb[:, sl], start=True, stop=True)
                else:
                    nc.tensor.matmul(a_ps, wf(k), y_sb[:, k - 1, :], start=True, stop=True)
                nc.scalar.activation(
                    y_sb[:, k, :], a_ps, AF.Tanh, bias=bias_sb[:, k : k + 1]
                )
                heartbeat(wf(max(k, 1)), y_sb[:, k, :])
                sq = work.tile([W, CHUNK], BF16, tag="sq")
                nc.gpsimd.tensor_tensor(sq, y_sb[:, k, :], y_sb[:, k, :], ALU.mult)
                nc.vector.tensor_scalar(
                    d_sb[:, k, :], sq, -1.0, 1.0, ALU.mult, ALU.add
                )

            if stage == "fwd":
                res_sb = work.tile([1, CHUNK], F32, tag="res")
                nc.vector.tensor_copy(res_sb, y_sb[0:1, NLAY - 1, :])
                nc.sync.dma_start(out=resid[c : c + 1, :], in_=res_sb[:])
                continue

            # ---- backward ----
            vt = work.tile([W, CHUNK], BF16, tag="vt")
            nc.vector.tensor_scalar_mul(vt, d_sb[:, NLAY - 1, :], wout_sb)
            nc.gpsimd.tensor_tensor(
                r_sb[:, NLAY - 1, :], y_sb[:, NLAY - 1, :], vt, ALU.mult
            )
            for k in range(NLAY - 1, 0, -1):
                v_ps = ps_f.tile([W, CHUNK], F32, tag="a")
                nc.tensor.matmul(v_ps, wb(k), vt, start=True, stop=True)
                vt = work.tile([W, CHUNK], BF16, tag="vt")
                nc.vector.tensor_tensor(vt, d_sb[:, k - 1, :], v_ps, ALU.mult)
                heartbeat(wb(k), vt)
                nc.gpsimd.tensor_tensor(
                    r_sb[:, k - 1, :], y_sb[:, k - 1, :], vt, ALU.mult
                )

            if stage == "bwd":
                res_sb = work.tile([1, CHUNK], F32, tag="res")
                nc.vector.tensor_copy(res_sb, r_sb[0:1, 0, :])
                nc.sync.dma_start(out=resid[c : c + 1, :], in_=res_sb[:])
                continue

            # ---- psi network (independent; scheduler slots it into gaps) ----
            pp_ps = ps_f.tile([PW, CHUNK], F32, tag="a")
            nc.tensor.matmul(pp_ps, pw1t_sb, pts_sb[:, sl], start=True, stop=True)
            hp1 = work.tile([PW, CHUNK], BF16, tag="hp")
            nc.scalar.activation(hp1, pp_ps, AF.Tanh, bias=pbias3_sb[:, 0:1])
            pp2_ps = ps_f.tile([PW, CHUNK], F32, tag="a")
            nc.tensor.matmul(pp2_ps, pw2t_sb, hp1, start=True, stop=True)
            hp2 = work.tile([PW, CHUNK], BF16, tag="hp")
            nc.scalar.activation(hp2, pp2_ps, AF.Tanh, bias=pbias3_sb[:, 1:2])
            psi_ps = ps_f.tile([1, CHUNK], F32, tag="a")
            nc.tensor.matmul(psi_ps, pwot_sb, hp2, start=True, stop=True)
            c2 = work.tile([1, CHUNK], F32, tag="c2")
            nc.scalar.activation(c2, psi_ps, AF.Square, bias=pb1_sb)

            # ---- jets + curvature contraction ----
            # acc row 0 = u_tt, row 32 = lap (matmul outs need base part 0/32/64)
            # two independent direction-pair pipelines: half 0 = (t,x), 1 = (y,z)
            acc_ps = ps_d.tile([33, CHUNK], F32, tag="acc")
            nc.tensor.matmul(
                acc_ps[0:1, :], jl2_sb[:, 0:1], r_sb[:, 0, :], start=True, stop=False,
                skip_group_check=True,
            )
            nc.tensor.matmul(
                acc_ps[32:33, :], jl2_sb[:, 1:2], r_sb[:, 0, :], start=True, stop=False,
                skip_group_check=True,
            )
            hjh = []
            for h in range(2):
                hj = jwork.tile([W, 2, CHUNK], BF16, tag="hj")
                for i in range(2):
                    nc.vector.tensor_scalar_mul(
                        hj[:, i, :], d_sb[:, 0, :],
                        w1cols_sb[:, 2 * h + i : 2 * h + i + 1],
                    )
                hjh.append(hj)
            for k in range(1, NLAY):
                last = k == NLAY - 1
                for h in range(2):
                    aj_ps = ps_j.tile([W, 2, CHUNK], F32, tag="aj")
                    for i in range(2):
                        nc.tensor.matmul(
                            aj_ps[:, i, :], wf(k), hjh[h][:, i, :],
                            start=True, stop=True,
                        )
                    sqj = jwork.tile([W, 2, CHUNK], BF16, tag="sqj")
                    nc.scalar.activation(sqj, aj_ps, AF.Square)
                    heartbeat(wf(k), sqj[:, 0, :])
                    if not last:
                        hj = jwork.tile([W, 2, CHUNK], BF16, tag="hj")
                        dbc = d_sb[:, k, :].unsqueeze(1).to_broadcast((W, 2, CHUNK))
                        nc.vector.tensor_tensor(hj, dbc, aj_ps, ALU.mult)
                        hjh[h] = hj
                    # mall[:,i,:] = sqj[:,i,:] * r_k  (fused, r broadcast)
                    mall = jwork.tile([W, 2, CHUNK], BF16, tag="mall")
                    rbc = r_sb[:, k, :].unsqueeze(1).to_broadcast((W, 2, CHUNK))
                    nc.vector.tensor_tensor(mall, sqj, rbc, ALU.mult)
                    if h == 0:
                        nc.tensor.matmul(
                            acc_ps[0:1, :], ones_sb, mall[:, 0, :],
                            start=False, stop=last, skip_group_check=True,
                        )
                        nc.tensor.matmul(
                            acc_ps[32:33, :], ones_sb, mall[:, 1, :],
                            start=False, stop=False, skip_group_check=True,
                        )
                    else:
                        for i in range(2):
                            nc.tensor.matmul(
                                acc_ps[32:33, :], ones_sb, mall[:, i, :],
                                start=False, stop=(last and i == 1),
                                skip_group_check=True,
                            )

            if stage == "jets":
                res_sb = work.tile([1, CHUNK], F32, tag="res")
                nc.vector.tensor_copy(res_sb, acc_ps[0:1, :])
                nc.sync.dma_start(out=resid[c : c + 1, :], in_=res_sb[:])
                continue

            # ---- tail: resid = dt - c2*dlap ----
            m1 = work.tile([1, CHUNK], F32, tag="m1")
            nc.vector.scalar_tensor_tensor(
                m1, acc_ps[32:33, :], -1.0, c2, ALU.mult, ALU.mult
            )
            res_sb = work.tile([1, CHUNK], F32, tag="res")
            nc.vector.tensor_tensor(res_sb, m1, acc_ps[0:1, :], ALU.add)
            nc.sync.dma_start(out=resid[c : c + 1, :], in_=res_sb[:])

    return nc


_NC_CACHE = {}


def _get_nc():
    if "nc" not in _NC_CACHE:
        nc = build_nc()
        nc.finalize()
        _NC_CACHE["nc"] = nc
    return _NC_CACHE["nc"]


def _bf(a):
    import ml_dtypes

    return np.asarray(a, np.float32).astype(ml_dtypes.bfloat16)


def make_in_maps(t, x, y, z, uW_in, ub_in, uW_hid, ub_hid, uW_out, ub_out,
                 pW_in, pb_in, pW_hid, pb_hid, pW_out, pb_out):
    f = lambda a: np.ascontiguousarray(np.asarray(a, np.float32))
    uW_in, ub_in, uW_hid, ub_hid = f(uW_in), f(ub_in), f(uW_hid), f(ub_hid)
    uW_out, pW_in, pb_in = f(uW_out), f(pW_in), f(pb_in)
    pW_hid, pb_hid, pW_out, pb_out = f(pW_hid), f(pb_hid), f(pW_out), f(pb_out)

    pts_full = np.stack([f(t), f(x), f(y), f(z)], axis=0)  # [4, 16384]

    # wpf: biases [W,6] | wout [W,1] | w1cols [W,4]
    biases = np.concatenate([ub_in[:, None], ub_hid.T], axis=1)
    wpf = np.concatenate([biases, uW_out[0][:, None], uW_in], axis=1)
    # wpb: wfwd [W,640] | wbwd [W,640] | jl2 [W,2] | ones [W,1]
    wfwd = np.concatenate([uW_hid[i].T for i in range(NHID)], axis=1)
    wbwd = np.concatenate([uW_hid[i] for i in range(NHID)], axis=1)
    jl2 = -2.0 * np.stack([uW_in[:, 0] ** 2, (uW_in[:, 1:4] ** 2).sum(1)], axis=1)
    wpb = np.concatenate([wfwd, wbwd, jl2, -2.0 * np.ones([W, 1], np.float32)], axis=1)
    # ppack: w1t [4,128] | pw1t [4,32]
    ppk = np.concatenate([uW_in.T, pW_in.T], axis=1)
    # pwpack: pw2t [32,32] | pwot [32,1]
    pwp = np.concatenate([pW_hid[0].T, pW_out[0][:, None]], axis=1)
    # pbias3: pb_in | pb_hid[0] | (pb_out+1 at row 0)
    pb3 = np.zeros([PW, 3], np.float32)
    pb3[:, 0] = pb_in
    pb3[:, 1] = pb_hid[0]
    pb3[0, 2] = pb_out[0] + 1.0

    shared = dict(
        wpf=f(wpf),
        wpb=_bf(wpb),
        ppack=f(ppk),
        pwpack=_bf(pwp),
        pbias3=f(pb3),
    )
    in_maps = []
    for cid in range(N_CORES):
        m = dict(shared)
        m["pts"] = np.ascontiguousarray(pts_full[:, cid * NPTS : (cid + 1) * NPTS])
        in_maps.append(m)
    return in_maps


def kernel(**inputs):
    in_maps = make_in_maps(**inputs)
    nc = _get_nc()
    res = run_bass_kernel_spmd(nc, in_maps, list(range(N_CORES))).results
    out = np.concatenate(
        [np.asarray(res[cid]["resid"]).reshape(-1) for cid in range(N_CORES)]
    )
    return out.astype(np.float32)


if __name__ == "__main__":
    nc = build_nc()
    print("built ok:", nc)
